# revision 12
# baseline (speedup 1.0000x reference)
"""Bigram LM loss kernel for 8 Trainium2 NeuronCores.

reference:
    x = emb[idx]                         # [B,S,D] gather
    logits = einsum('bsd,vd->bsv', x, W) + b
    loss = -mean(log_softmax(logits)[targets])
    returns (logits, loss)

Strategy (vocab/tensor parallel, per the lm_head sharding):
  - Pad V=50257 -> 50264 = 8*6283; core k owns vocab slice [k*6283,(k+1)*6283).
  - W is pre-transposed, bf16-cast and pre-swizzled into the SBUF layout on
    the host -> each core keeps its W_T shard resident in SBUF (~100KB per
    partition); matmul runs in bf16 with f32 PSUM accumulation.
  - emb is bf16-cast on host; x rows are gathered on-device by indirect DMA
    (128 rows/tile) and transposed to [d, t] layout with the DMA xbar.
  - W streams in 13 ~1MB chunks so compute starts after the first chunk;
    while it streams, the PE broadcasts the bias across partitions with
    K=1 ones-matmuls (which also warms the HAM clock gate).
  - Per 128-token tile: 13 vocab tiles x 8 K-tiles of matmul; DVE adds the
    bias while draining PSUM->SBUF; ACT computes exp(logits) with a fused
    per-token row-sum (accum_out) for the softmax denominator.
  - Host gathers the 8 logits shards, combines per-token sum-exp partials
    (the "all-reduce" on the per-token logsumexp), extracts target logits,
    and forms the scalar loss.
"""

import numpy as np
import ml_dtypes

import concourse.bass as bass
import concourse.mybir as mybir
import concourse.tile as tile
from concourse import bacc
from concourse.bass_utils import run_bass_kernel_spmd

# Problem constants (hardcoded per contract)
V = 50257
D = 1024
B, S = 4, 2048
T = B * S  # 8192 tokens
NCORES = 8
VS = 6283  # per-core vocab shard (V padded to 50264)
VP = VS * NCORES
P = 128
TT = T // P  # 64 token tiles
KT = D // P  # 8 contraction tiles
NV = 512  # PSUM bank capacity in f32
# Vocab tiling: 13x448 + 459. 448 keeps SBUF slice strides 64B-aligned
# (448*2B = 14*64B) and every tile's streaming time (>=187ns) hides the
# ~95ns LDWEIGHTS (a 139-wide tail tile would expose it).
_sizes = [448] * 13 + [459]
NJ = len(_sizes)
VTILES = []
_v = 0
for _n in _sizes:
    VTILES.append((_v, _n))
    _v += _n
assert _v == VS
# packed per-partition W layout: [j | k | c]
WOFFS = []
_o = 0
for _v0, _n in VTILES:
    WOFFS.append(_o)
    _o += KT * _n
WTOT = _o  # 50264
PAD_BIAS = -30000.0  # exp() underflows to 0; padded cols are discarded on host

BF16 = mybir.dt.bfloat16
F32 = mybir.dt.float32

LAST_RESULT = None  # stashed BassKernelResults for the test harness

_CACHED_NC = None


def _build_nc():
    nc = bacc.Bacc("TRN2", target_bir_lowering=False, debug=False, num_devices=NCORES)

    wt = nc.dram_tensor("wt", [P, WTOT], BF16, kind="ExternalInput").ap()
    emb = nc.dram_tensor("emb", [V, D], BF16, kind="ExternalInput").ap()
    idxt = nc.dram_tensor("idxt", [P, TT], mybir.dt.int32, kind="ExternalInput").ap()
    bias = nc.dram_tensor("bias", [1, VS], BF16, kind="ExternalInput").ap()

    logits = nc.dram_tensor("logits", [T, VS], F32, kind="ExternalOutput").ap()
    sume = nc.dram_tensor("sume", [P, TT], F32, kind="ExternalOutput").ap()

    with tile.TileContext(nc) as tc:
        with (
            tc.tile_pool(name="persist", bufs=1) as persist,
            tc.tile_pool(name="xgp", bufs=2) as xgp,
            tc.tile_pool(name="xtp", bufs=2) as xtp,
            tc.tile_pool(name="psp", bufs=8, space="PSUM") as psp,
            tc.tile_pool(name="lrp", bufs=2) as lrp,
            tc.tile_pool(name="etp", bufs=2) as etp,
            tc.tile_pool(name="secp", bufs=2) as secp,
        ):
            # --- one-time loads, in consumption order (sync HWDGE ring) ---
            idx_sb = persist.tile([P, TT], mybir.dt.int32)
            nc.sync.dma_start(idx_sb[:], idxt[:])
            bias_sb = persist.tile([1, VS], BF16)
            nc.sync.dma_start(bias_sb[:], bias[:])
            # W shard: one ~1MB contiguous DMA per vocab tile j (host-swizzled
            # [j | k | c] layout), so matmuls start after ~1MB instead of the
            # full 12.9MB stream.
            wsb = persist.tile([P, WTOT], BF16)
            for (v0, n), off in zip(VTILES, WOFFS):
                nc.sync.dma_start(wsb[:, off : off + KT * n], wt[:, off : off + KT * n])

            # Bias partition-broadcast via K=1 ones-matmuls on the PE. Two run
            # upfront (warms the HAM clock gate before the first real matmul);
            # the rest interleave into tile 0's loop, filling the PE bubbles
            # left by the W stream still arriving.
            ones_sb = persist.tile([1, P], BF16)
            nc.gpsimd.memset(ones_sb[:], 1.0)
            bb = persist.tile([P, VS], F32)

            def bcast_bias(j):
                v0, n = VTILES[j]
                bps = psp.tile([P, NV], F32, tag="ps", name=f"bps{j}")
                nc.tensor.matmul(
                    bps[:, :n],
                    ones_sb[:, :],
                    bias_sb[:, v0 : v0 + n],
                    start=True,
                    stop=True,
                )
                nc.vector.tensor_copy(bb[:, v0 : v0 + n], bps[:, :n])

            bcast_bias(0)
            bcast_bias(1)

            sume_sb = persist.tile([P, TT], F32)

            for i in range(TT):
                # gather this tile's 128 embedding rows: xg[p, :] = emb[idx[p], :]
                xg = xgp.tile([P, D], BF16, tag="xg")
                nc.gpsimd.indirect_dma_start(
                    out=xg[:],
                    out_offset=None,
                    in_=emb[:],
                    in_offset=bass.IndirectOffsetOnAxis(
                        ap=idx_sb[:, i : i + 1], axis=0
                    ),
                )
                # xbar transpose to lhsT layout: xt[p, k, t] = xg[t, k*128+p]
                # (dedicated scalar HWDGE ring)
                xt = xtp.tile([P, KT, P], BF16, tag="xt")
                nc.scalar.dma_start(xt[:], xg[:], transpose=True)

                lrow = lrp.tile([P, VS], F32, tag="lrow")
                sec = secp.tile([P, NJ], F32, tag="sec")
                last = i == TT - 1
                for j, (v0, n) in enumerate(VTILES):
                    ps = psp.tile([P, NV], F32, tag="ps")
                    for k in range(KT):
                        wk = WOFFS[j] + k * n
                        nc.tensor.matmul(
                            ps[:, :n],
                            xt[:, k, :],
                            wsb[:, wk : wk + n],
                            start=(k == 0),
                            stop=(k == KT - 1),
                        )
                    # drain PSUM -> SBUF with fused bias add
                    nc.vector.tensor_add(
                        lrow[:, v0 : v0 + n], ps[:, :n], bb[:, v0 : v0 + n]
                    )
                    # exp with fused per-token row-sum (softmax denominator)
                    et = etp.tile([P, NV], F32, tag="et")
                    nc.scalar.activation(
                        et[:, :n],
                        lrow[:, v0 : v0 + n],
                        mybir.ActivationFunctionType.Exp,
                        accum_out=sec[:, j : j + 1],
                    )
                    if last:
                        # chunked writes overlap the final tile's compute
                        nc.sync.dma_start(
                            logits[i * P : (i + 1) * P, v0 : v0 + n],
                            lrow[:, v0 : v0 + n],
                        )
                    if i == 0 and j + 2 < NJ:
                        bcast_bias(j + 2)
                if not last:
                    nc.sync.dma_start(logits[i * P : (i + 1) * P, :], lrow[:])
                nc.vector.tensor_reduce(
                    sume_sb[:, i : i + 1],
                    sec[:],
                    axis=mybir.AxisListType.X,
                    op=mybir.AluOpType.add,
                )
            nc.sync.dma_start(sume[:], sume_sb[:])

    nc.compile()
    return nc


def _swizzle_wt(wt_core):
    """[1024, VS] -> packed [128, WTOT] with per-partition [j | k | c] layout."""
    a3 = wt_core.reshape(KT, P, VS)  # [k, p, c]
    chunks = [
        a3[:, :, v0 : v0 + n].transpose(1, 0, 2).reshape(P, KT * n)
        for v0, n in VTILES
    ]
    return np.ascontiguousarray(np.concatenate(chunks, axis=1))


def kernel(idx, targets, emb, W, b):
    global LAST_RESULT, _CACHED_NC

    idx = np.asarray(idx).astype(np.int32).reshape(T)
    targets = np.asarray(targets).astype(np.int64).reshape(T)
    emb = np.asarray(emb, dtype=np.float32)
    W = np.asarray(W, dtype=np.float32)
    b = np.asarray(b, dtype=np.float32)

    # ---- host-side input prep (sharding + layout) ----
    emb_bf = emb.astype(ml_dtypes.bfloat16)
    wt_full = np.zeros((D, VP), dtype=ml_dtypes.bfloat16)
    wt_full[:, :V] = np.ascontiguousarray(W.T).astype(ml_dtypes.bfloat16)
    b_pad = np.full(VP, PAD_BIAS, dtype=np.float32)
    b_pad[:V] = b
    idxt = np.ascontiguousarray(idx.reshape(TT, P).T)  # [P, TT] int32

    in_maps = []
    for k in range(NCORES):
        sl = slice(k * VS, (k + 1) * VS)
        in_maps.append(
            {
                "wt": _swizzle_wt(wt_full[:, sl]),
                "emb": emb_bf,
                "idxt": idxt,
                "bias": b_pad[sl].reshape(1, VS).astype(ml_dtypes.bfloat16),
            }
        )

    if _CACHED_NC is None:
        _CACHED_NC = _build_nc()
    nc = _CACHED_NC

    res = run_bass_kernel_spmd(nc, in_maps, core_ids=list(range(NCORES)))
    LAST_RESULT = res

    # ---- host-side unshard / combine ----
    logits_flat = np.empty((T, V), dtype=np.float32)
    sumexp = np.zeros(T, dtype=np.float64)
    for k in range(NCORES):
        r = res.results[k]
        v0 = k * VS
        w = min(VS, V - v0)
        logits_flat[:, v0 : v0 + w] = r["logits"][:, :w]
        # sume[p, i] is token i*128+p
        sumexp += r["sume"].T.reshape(T).astype(np.float64)

    logsumexp = np.log(sumexp)  # [T]
    tgt_logit = logits_flat[np.arange(T), targets].astype(np.float64)
    loss = np.float32(np.mean(logsumexp - tgt_logit))

    logits_out = logits_flat.reshape(B, S, V)
    return logits_out, loss


# revision 13
# speedup vs baseline: 1.0035x; 1.0035x over previous
"""Bigram LM loss kernel for 8 Trainium2 NeuronCores.

reference:
    x = emb[idx]                         # [B,S,D] gather
    logits = einsum('bsd,vd->bsv', x, W) + b
    loss = -mean(log_softmax(logits)[targets])
    returns (logits, loss)

Strategy (vocab/tensor parallel, per the lm_head sharding):
  - Pad V=50257 -> 50264 = 8*6283; core k owns vocab slice [k*6283,(k+1)*6283).
  - W is pre-transposed, bf16-cast and pre-swizzled into the SBUF layout on
    the host -> each core keeps its W_T shard resident in SBUF (~100KB per
    partition); matmul runs in bf16 with f32 PSUM accumulation.
  - emb is bf16-cast on host; x rows are gathered on-device by indirect DMA
    (128 rows/tile) and transposed to [d, t] layout with the DMA xbar.
  - W streams in 13 ~1MB chunks so compute starts after the first chunk;
    while it streams, the PE broadcasts the bias across partitions with
    K=1 ones-matmuls (which also warms the HAM clock gate).
  - Per 128-token tile: 13 vocab tiles x 8 K-tiles of matmul; DVE adds the
    bias while draining PSUM->SBUF; ACT computes exp(logits) with a fused
    per-token row-sum (accum_out) for the softmax denominator.
  - Host gathers the 8 logits shards, combines per-token sum-exp partials
    (the "all-reduce" on the per-token logsumexp), extracts target logits,
    and forms the scalar loss.
"""

import numpy as np
import ml_dtypes

import concourse.bass as bass
import concourse.mybir as mybir
import concourse.tile as tile
from concourse import bacc
from concourse.bass_utils import run_bass_kernel_spmd

# Problem constants (hardcoded per contract)
V = 50257
D = 1024
B, S = 4, 2048
T = B * S  # 8192 tokens
NCORES = 8
VS = 6283  # per-core vocab shard (V padded to 50264)
VP = VS * NCORES
P = 128
TT = T // P  # 64 token tiles
KT = D // P  # 8 contraction tiles
NV = 512  # PSUM bank capacity in f32
# Vocab tiling: 13x448 + 459. 448 keeps SBUF slice strides 64B-aligned
# (448*2B = 14*64B) and every tile's streaming time (>=187ns) hides the
# ~95ns LDWEIGHTS (a 139-wide tail tile would expose it).
_sizes = [448] * 13 + [459]
NJ = len(_sizes)
VTILES = []
_v = 0
for _n in _sizes:
    VTILES.append((_v, _n))
    _v += _n
assert _v == VS
# packed per-partition W layout: [j | k | c]
WOFFS = []
_o = 0
for _v0, _n in VTILES:
    WOFFS.append(_o)
    _o += KT * _n
WTOT = _o  # 50264
PAD_BIAS = -30000.0  # exp() underflows to 0; padded cols are discarded on host

BF16 = mybir.dt.bfloat16
F32 = mybir.dt.float32

LAST_RESULT = None  # stashed BassKernelResults for the test harness

_CACHED_NC = None


def _build_nc():
    nc = bacc.Bacc("TRN2", target_bir_lowering=False, debug=False, num_devices=NCORES)

    wt = nc.dram_tensor("wt", [P, WTOT], BF16, kind="ExternalInput").ap()
    emb = nc.dram_tensor("emb", [V, D], BF16, kind="ExternalInput").ap()
    idxt = nc.dram_tensor("idxt", [P, TT], mybir.dt.int32, kind="ExternalInput").ap()
    bias = nc.dram_tensor("bias", [1, VS], BF16, kind="ExternalInput").ap()

    logits = nc.dram_tensor("logits", [T, VS], F32, kind="ExternalOutput").ap()
    sume = nc.dram_tensor("sume", [P, TT], F32, kind="ExternalOutput").ap()

    with tile.TileContext(nc) as tc:
        with (
            tc.tile_pool(name="persist", bufs=1) as persist,
            tc.tile_pool(name="xgp", bufs=2) as xgp,
            tc.tile_pool(name="xtp", bufs=2) as xtp,
            tc.tile_pool(name="psp", bufs=8, space="PSUM") as psp,
            tc.tile_pool(name="lrp", bufs=2) as lrp,
            tc.tile_pool(name="etp", bufs=2) as etp,
            tc.tile_pool(name="secp", bufs=2) as secp,
        ):
            # --- one-time loads, in consumption order (sync HWDGE ring) ---
            idx_sb = persist.tile([P, TT], mybir.dt.int32)
            nc.sync.dma_start(idx_sb[:], idxt[:])
            bias_sb = persist.tile([1, VS], BF16)
            nc.sync.dma_start(bias_sb[:], bias[:])
            # W shard: one ~1MB contiguous DMA per vocab tile j (host-swizzled
            # [j | k | c] layout), so matmuls start after ~1MB instead of the
            # full 12.9MB stream.
            wsb = persist.tile([P, WTOT], BF16)
            for (v0, n), off in zip(VTILES, WOFFS):
                nc.sync.dma_start(wsb[:, off : off + KT * n], wt[:, off : off + KT * n])

            # Bias partition-broadcast via K=1 ones-matmuls on the PE. Two run
            # upfront (warms the HAM clock gate before the first real matmul);
            # the rest interleave into tile 0's loop, filling the PE bubbles
            # left by the W stream still arriving.
            ones_sb = persist.tile([1, P], BF16)
            nc.gpsimd.memset(ones_sb[:], 1.0)
            bb = persist.tile([P, VS], F32)

            def bcast_bias(j):
                v0, n = VTILES[j]
                bps = psp.tile([P, NV], F32, tag="ps", name=f"bps{j}")
                nc.tensor.matmul(
                    bps[:, :n],
                    ones_sb[:, :],
                    bias_sb[:, v0 : v0 + n],
                    start=True,
                    stop=True,
                )
                nc.vector.tensor_copy(bb[:, v0 : v0 + n], bps[:, :n])

            bcast_bias(0)
            bcast_bias(1)

            sume_sb = persist.tile([P, TT], F32)

            tiles = {}  # i -> (xt, lrow, sec)

            def start_tile(i):
                # gather this tile's 128 embedding rows: xg[p, :] = emb[idx[p], :]
                xg = xgp.tile([P, D], BF16, tag="xg", name=f"xg{i}")
                nc.gpsimd.indirect_dma_start(
                    out=xg[:],
                    out_offset=None,
                    in_=emb[:],
                    in_offset=bass.IndirectOffsetOnAxis(
                        ap=idx_sb[:, i : i + 1], axis=0
                    ),
                )
                # xbar transpose to lhsT layout: xt[p, k, t] = xg[t, k*128+p]
                # (dedicated scalar HWDGE ring)
                xt = xtp.tile([P, KT, P], BF16, tag="xt", name=f"xt{i}")
                nc.scalar.dma_start(xt[:], xg[:], transpose=True)
                lrow = lrp.tile([P, VS], F32, tag="lrow", name=f"lrow{i}")
                sec = secp.tile([P, NJ], F32, tag="sec", name=f"sec{i}")
                tiles[i] = (xt, lrow, sec)

            def group(i, j, chunked_out):
                xt, lrow, sec = tiles[i]
                v0, n = VTILES[j]
                ps = psp.tile([P, NV], F32, tag="ps", name=f"ps{i}_{j}")
                for k in range(KT):
                    wk = WOFFS[j] + k * n
                    nc.tensor.matmul(
                        ps[:, :n],
                        xt[:, k, :],
                        wsb[:, wk : wk + n],
                        start=(k == 0),
                        stop=(k == KT - 1),
                    )
                # drain PSUM -> SBUF with fused bias add
                nc.vector.tensor_add(
                    lrow[:, v0 : v0 + n], ps[:, :n], bb[:, v0 : v0 + n]
                )
                # exp with fused per-token row-sum (softmax denominator)
                et = etp.tile([P, NV], F32, tag="et", name=f"et{i}_{j}")
                nc.scalar.activation(
                    et[:, :n],
                    lrow[:, v0 : v0 + n],
                    mybir.ActivationFunctionType.Exp,
                    accum_out=sec[:, j : j + 1],
                )
                if chunked_out:
                    # chunked writes overlap this tile's remaining compute
                    nc.sync.dma_start(
                        logits[i * P : (i + 1) * P, v0 : v0 + n],
                        lrow[:, v0 : v0 + n],
                    )

            def finish_tile(i, chunked_out):
                _, lrow, sec = tiles.pop(i)
                if not chunked_out:
                    nc.sync.dma_start(logits[i * P : (i + 1) * P, :], lrow[:])
                nc.vector.tensor_reduce(
                    sume_sb[:, i : i + 1],
                    sec[:],
                    axis=mybir.AxisListType.X,
                    op=mybir.AluOpType.add,
                )

            # Tiles 0 and 1 run staggered-interleaved: two tiles consume each
            # newly arrived W chunk, so the group rate (~2x1.7us per chunk)
            # stays below the W stream's ~3us/chunk arrival rate -> no PE
            # stalls while W streams in. Remaining bias broadcasts fill the
            # tile-0-only prefix.
            STAG = 4
            start_tile(0)
            start_tile(1)
            for j in range(STAG):
                group(0, j, False)
                bcast_bias(j + 2)
            for j in range(STAG, NJ):
                group(0, j, False)
                if j + 2 < NJ:
                    bcast_bias(j + 2)
                group(1, j - STAG, False)
            finish_tile(0, False)
            for j in range(NJ - STAG, NJ):
                group(1, j, False)
            finish_tile(1, False)

            for i in range(2, TT):
                start_tile(i)
                last = i == TT - 1
                for j in range(NJ):
                    group(i, j, last)
                finish_tile(i, last)
            nc.sync.dma_start(sume[:], sume_sb[:])

    nc.compile()
    return nc


def _swizzle_wt(wt_core):
    """[1024, VS] -> packed [128, WTOT] with per-partition [j | k | c] layout."""
    a3 = wt_core.reshape(KT, P, VS)  # [k, p, c]
    chunks = [
        a3[:, :, v0 : v0 + n].transpose(1, 0, 2).reshape(P, KT * n)
        for v0, n in VTILES
    ]
    return np.ascontiguousarray(np.concatenate(chunks, axis=1))


def kernel(idx, targets, emb, W, b):
    global LAST_RESULT, _CACHED_NC

    idx = np.asarray(idx).astype(np.int32).reshape(T)
    targets = np.asarray(targets).astype(np.int64).reshape(T)
    emb = np.asarray(emb, dtype=np.float32)
    W = np.asarray(W, dtype=np.float32)
    b = np.asarray(b, dtype=np.float32)

    # ---- host-side input prep (sharding + layout) ----
    emb_bf = emb.astype(ml_dtypes.bfloat16)
    wt_full = np.zeros((D, VP), dtype=ml_dtypes.bfloat16)
    wt_full[:, :V] = np.ascontiguousarray(W.T).astype(ml_dtypes.bfloat16)
    b_pad = np.full(VP, PAD_BIAS, dtype=np.float32)
    b_pad[:V] = b
    idxt = np.ascontiguousarray(idx.reshape(TT, P).T)  # [P, TT] int32

    in_maps = []
    for k in range(NCORES):
        sl = slice(k * VS, (k + 1) * VS)
        in_maps.append(
            {
                "wt": _swizzle_wt(wt_full[:, sl]),
                "emb": emb_bf,
                "idxt": idxt,
                "bias": b_pad[sl].reshape(1, VS).astype(ml_dtypes.bfloat16),
            }
        )

    if _CACHED_NC is None:
        _CACHED_NC = _build_nc()
    nc = _CACHED_NC

    res = run_bass_kernel_spmd(nc, in_maps, core_ids=list(range(NCORES)))
    LAST_RESULT = res

    # ---- host-side unshard / combine ----
    logits_flat = np.empty((T, V), dtype=np.float32)
    sumexp = np.zeros(T, dtype=np.float64)
    for k in range(NCORES):
        r = res.results[k]
        v0 = k * VS
        w = min(VS, V - v0)
        logits_flat[:, v0 : v0 + w] = r["logits"][:, :w]
        # sume[p, i] is token i*128+p
        sumexp += r["sume"].T.reshape(T).astype(np.float64)

    logsumexp = np.log(sumexp)  # [T]
    tgt_logit = logits_flat[np.arange(T), targets].astype(np.float64)
    loss = np.float32(np.mean(logsumexp - tgt_logit))

    logits_out = logits_flat.reshape(B, S, V)
    return logits_out, loss
